# revision 7
# baseline (speedup 1.0000x reference)
"""Trainium2 Bass kernel for a 3-layer GraphSAGE+SAGPool GNN.

Strategy: the dominant cost (~97% of FLOPs and memory traffic) is layer 0,
which touches x [50000, 1036] (207MB).  By linearity of matmul over the
mean-aggregation, we compute y = x @ [wl|wr] FIRST (one fused 1036->256
matmul), then aggregate in 128-dim space via indirect-DMA gather + one-hot
scatter-matmuls.  The device outputs h0 = relu(mean@wl + bl + x@wr) for all
nodes; the host (numpy) finishes the cheap remainder (SAG pooling, layers
1-2 on <=10k nodes, readouts, patient pooling, dense heads).

Sharding: data-parallel over graphs.  8 cores x 7 graph slots of 1024
padded nodes each.  Each graph's edges are local to its core (checked; a
pure-numpy fallback handles arbitrary inputs).
"""

import os
import sys

import numpy as np

# ---- problem constants (hardcoded per harness contract) ----
G, N0, P = 50, 1000, 10
FEAT, NHID, GDIM, RATIO = 1036, 128, 32, 0.2
NLAYERS = 3
NCORES = 8
SLOTS = 7            # graph slots per core (7*8 = 56 >= 50)
NPG = 1024           # padded nodes per graph slot
TPG = NPG // 128     # node-tiles per graph slot (8)
M_PAD = SLOTS * NPG  # padded nodes per core (7168)
NTILES = SLOTS * TPG # node-tiles per core (56)

_TRN_REPO = "/opt/trn_rl_repo"

_nc_cache = {}
_last_exec_time_ns = None


# --------------------------------------------------------------------------
# host-side numpy ops mirroring the jax reference
# --------------------------------------------------------------------------

def _relu(v):
    return np.maximum(v, np.float32(0.0))


def _log_softmax(v):
    m = v.max(axis=1, keepdims=True)
    s = v - m
    return s - np.log(np.exp(s).sum(axis=1, keepdims=True))


def _sigmoid(v):
    return 1.0 / (1.0 + np.exp(-v))


def _topk_idx(score, k):
    # lax.top_k: descending values, ties -> smaller index first
    return np.argsort(-score, axis=1, kind="stable")[:, :k]


def _finish_from_h0(h0, src, dst, graphs_per_pat, prm):
    """Everything after the device-computed layer-0 SAGE output h0."""
    f32 = np.float32
    x = h0.astype(f32, copy=False)
    n = N0
    src_c = src.astype(np.int64, copy=False)
    dst_c = dst.astype(np.int64, copy=False)
    xs = []
    for l in range(NLAYERS):
        k = int(np.ceil(RATIO * n))
        M = G * n
        if l > 0:
            # SAGE (mean aggr) on compacted surviving edges
            yl = x @ prm[f"c{l}_wl"]
            yr = x @ prm[f"c{l}_wr"]
            agg = np.zeros((M, NHID), f32)
            np.add.at(agg, dst_c, yl[src_c])
            cnt = np.bincount(dst_c, minlength=M).astype(f32)
            x = _relu(agg / np.maximum(cnt, 1.0)[:, None] + prm[f"c{l}_bl"] + yr)
        # SAGPool: GraphConv (add aggr) score -> per-graph top-k
        s = x @ prm[f"p{l}_wrel"]  # [M, 1]
        aggs = np.zeros((M, 1), f32)
        np.add.at(aggs, dst_c, s[src_c])
        score = (aggs + prm[f"p{l}_brel"] + x @ prm[f"p{l}_wroot"]).reshape(G, n)
        idx = _topk_idx(score, k)
        sel = np.take_along_axis(score, idx, axis=1)
        perm = (idx + (np.arange(G) * n)[:, None]).reshape(-1)
        x = x[perm] * np.tanh(sel).reshape(-1, 1).astype(f32)
        mapping = np.full(M, -1, np.int64)
        mapping[perm] = np.arange(G * k)
        ns, nd = mapping[src_c], mapping[dst_c]
        keep = (ns >= 0) & (nd >= 0)
        src_c, dst_c = ns[keep], nd[keep]
        n = k
        xr = x.reshape(G, n, NHID)
        xs.append(np.concatenate([xr.max(axis=1), xr.mean(axis=1)], axis=1))
    h = xs[0] + xs[1] + xs[2]  # [G, 2*NHID]

    # patient-level mean pool (jnp.repeat with total_repeat_length=G)
    gpp = np.asarray(graphs_per_pat).astype(np.int64)
    bv = np.repeat(np.arange(P), np.maximum(gpp, 0))
    if len(bv) >= G:
        bv = bv[:G]
    else:
        pad_val = bv[-1] if len(bv) else 0
        bv = np.concatenate([bv, np.full(G - len(bv), pad_val, np.int64)])
    pooled = np.zeros((P, 2 * NHID), f32)
    np.add.at(pooled, bv, h)
    with np.errstate(divide="ignore", invalid="ignore"):
        pooled = pooled / gpp[:, None].astype(f32)

    h1 = _relu(pooled @ prm["lin1_w"] + prm["lin1_b"])
    feats = _relu(h1 @ prm["lin2_w"] + prm["lin2_b"])
    grade = _log_softmax(feats @ prm["grade_w"] + prm["grade_b"])
    hazard = _sigmoid(feats @ prm["haz_w"] + prm["haz_b"]) * 6.0 - 3.0
    return (
        feats.astype(f32),
        grade.astype(f32),
        hazard.astype(f32),
    )


def _layer0_host(x, src, dst, prm):
    """Pure-numpy layer-0 fallback (arbitrary edge structure)."""
    f32 = np.float32
    xn = x.copy()
    xn[:, :12] = xn[:, :12] / xn[:, :12].max(axis=0, keepdims=True)
    yl = xn @ prm["c0_wl"]
    yr = xn @ prm["c0_wr"]
    M = xn.shape[0]
    agg = np.zeros((M, NHID), f32)
    np.add.at(agg, dst.astype(np.int64), yl[src.astype(np.int64)])
    cnt = np.bincount(dst.astype(np.int64), minlength=M).astype(f32)
    return _relu(agg / np.maximum(cnt, 1.0)[:, None] + prm["c0_bl"] + yr)


def _install_ntff_hook():
    """Register the axon NTFF profiling hook (missing antenv.axon_hooks)."""
    import contextlib
    import ctypes
    import types

    if "antenv.axon_hooks" in sys.modules:
        return True
    try:
        import antenv
    except ImportError:
        return False
    so_path = "/opt/axon/libaxon_pjrt.so"
    if not os.path.exists(so_path):
        return False
    lib = ctypes.CDLL(so_path)
    if not hasattr(lib, "axon_start_nrt_profile"):
        return False
    lib.axon_start_nrt_profile.argtypes = [
        ctypes.POINTER(ctypes.c_int64), ctypes.c_size_t]
    lib.axon_start_nrt_profile.restype = ctypes.c_int64
    lib.axon_stop_nrt_profile.argtypes = [ctypes.c_char_p]
    lib.axon_stop_nrt_profile.restype = ctypes.c_int64

    @contextlib.contextmanager
    def _hook(output_dir, device_ids):
        import jax

        jax.devices()
        if device_ids:
            ids = (ctypes.c_int64 * len(device_ids))(*device_ids)
            rc = lib.axon_start_nrt_profile(ids, len(device_ids))
        else:
            rc = lib.axon_start_nrt_profile(None, 0)
        if rc != 0:
            raise RuntimeError(f"axon_start_nrt_profile rc={rc}")
        try:
            yield
        finally:
            n = lib.axon_stop_nrt_profile(str(output_dir).encode())
            print(f"ntff profile: {n} file(s) -> {output_dir}", file=sys.stderr)

    mod = types.ModuleType("antenv.axon_hooks")
    mod.get_axon_ntff_profile_hook = lambda: _hook
    mod.set_axon_ntff_profile_hook = lambda h: None
    sys.modules["antenv.axon_hooks"] = mod
    antenv.axon_hooks = mod
    return True


# --------------------------------------------------------------------------
# device program
# --------------------------------------------------------------------------

def _build_nc(n_et, col_start, et_total):
    if _TRN_REPO not in sys.path:
        sys.path.insert(0, _TRN_REPO)
    from contextlib import ExitStack

    from concourse import bacc, bass, mybir, tile

    f32 = mybir.dt.float32
    f16 = mybir.dt.float16
    f32r = mybir.dt.float32r
    i32 = mybir.dt.int32
    add_op = mybir.AluOpType.add
    sub_op = mybir.AluOpType.subtract
    eq_op = mybir.AluOpType.is_equal
    mult_op = mybir.AluOpType.mult

    ET = max(et_total, 1)
    nc = bacc.Bacc("TRN2", target_bir_lowering=False, debug=False,
                   num_devices=NCORES)
    xT = nc.dram_tensor("xT", [FEAT, M_PAD], f32, kind="ExternalInput").ap()
    Wc = nc.dram_tensor("Wc", [FEAT, 2 * NHID], f32, kind="ExternalInput").ap()
    SRC = nc.dram_tensor("SRC", [128, ET], i32, kind="ExternalInput").ap()
    DREL = nc.dram_tensor("DREL", [128, ET], f32, kind="ExternalInput").ap()
    RECIP = nc.dram_tensor("RECIP", [128, NTILES], f32, kind="ExternalInput").ap()
    IOTA = nc.dram_tensor("IOTA", [128, 128], f32, kind="ExternalInput").ap()
    BL = nc.dram_tensor("BL", [128, 128], f32, kind="ExternalInput").ap()
    H0 = nc.dram_tensor("H0", [M_PAD, NHID], f32, kind="ExternalOutput").ap()
    # per-graph-slot yl table: [node, 0:128]=hi (f32r), [node, 128:256]=lo
    ylg = [nc.dram_tensor(f"yl{s}", [NPG, 2 * NHID], f32r) for s in range(SLOTS)]

    n_kt = (FEAT + 127) // 128
    KT = [(kt * 128, min(128, FEAT - kt * 128)) for kt in range(n_kt)]

    with tile.TileContext(nc) as tc, ExitStack() as ctx:
        cpool = ctx.enter_context(tc.tile_pool(name="consts", bufs=1))
        wfpool = ctx.enter_context(tc.tile_pool(name="wf", bufs=2))
        wpool = ctx.enter_context(tc.tile_pool(name="w", bufs=2 * n_kt))
        xfpool = ctx.enter_context(tc.tile_pool(name="xf", bufs=3))
        xrpool = ctx.enter_context(tc.tile_pool(name="xr", bufs=2 * n_kt))
        xepool = ctx.enter_context(tc.tile_pool(name="xe", bufs=2 * n_kt))
        ypsum = ctx.enter_context(tc.tile_pool(name="ypsum", bufs=4, space="PSUM"))
        apsum = ctx.enter_context(tc.tile_pool(name="apsum", bufs=4, space="PSUM"))
        ysb = ctx.enter_context(tc.tile_pool(name="ysb", bufs=4))
        ylpool = ctx.enter_context(tc.tile_pool(name="ylt", bufs=4))
        yrpool = ctx.enter_context(tc.tile_pool(name="yrb", bufs=2 * TPG))
        gpool = ctx.enter_context(tc.tile_pool(name="gath", bufs=8))
        spool = ctx.enter_context(tc.tile_pool(name="sel", bufs=8))
        hpool = ctx.enter_context(tc.tile_pool(name="h0t", bufs=4))

        iota_sb = cpool.tile([128, 128], f32)
        nc.sync.dma_start(out=iota_sb[:], in_=IOTA[:])
        bl_sb = cpool.tile([128, 128], f32)
        nc.sync.dma_start(out=bl_sb[:], in_=BL[:])
        recip_sb = cpool.tile([128, NTILES], f32)
        nc.sync.dma_start(out=recip_sb[:], in_=RECIP[:])
        src_sb = cpool.tile([128, ET], i32)
        nc.sync.dma_start(out=src_sb[:], in_=SRC[:])
        drel_sb = cpool.tile([128, ET], f32)
        nc.sync.dma_start(out=drel_sb[:], in_=DREL[:])

        # W split: w = w1 + w2 (fp16 hi + fp16 residual), exact to ~2^-22
        w1_sb = []
        w2_sb = []
        for k0, kn in KT:
            wf = wfpool.tile([128, 2 * NHID], f32)
            nc.sync.dma_start(out=wf[:kn, :], in_=Wc[k0:k0 + kn, :])
            t1 = wpool.tile([128, 2 * NHID], f16, tag="w1")
            nc.vector.tensor_copy(out=t1[:kn, :], in_=wf[:kn, :])
            t2 = wpool.tile([128, 2 * NHID], f16, tag="w2")
            nc.vector.tensor_tensor(out=t2[:kn, :], in0=wf[:kn, :],
                                    in1=t1[:kn, :], op=sub_op)
            w1_sb.append(t1)
            w2_sb.append(t2)

        for s in range(SLOTS):
            # ---- phase 1: y = x @ [wl|wr] exactly via f32r split ----
            xrs, xes = [], []
            for kt, (k0, kn) in enumerate(KT):
                xf = xfpool.tile([128, NPG], f32)
                nc.sync.dma_start(out=xf[:kn, :],
                                  in_=xT[k0:k0 + kn, s * NPG:(s + 1) * NPG])
                xr = xrpool.tile([128, NPG], f16)
                nc.vector.tensor_copy(out=xr[:kn, :], in_=xf[:kn, :])
                xe = xepool.tile([128, NPG], f16)
                nc.vector.tensor_tensor(out=xe[:kn, :], in0=xf[:kn, :],
                                        in1=xr[:kn, :], op=sub_op)
                xrs.append(xr)
                xes.append(xe)
            yr_tiles = []
            for ntl in range(TPG):
                nsl = slice(ntl * 128, (ntl + 1) * 128)
                py = ypsum.tile([128, 2 * NHID], f32)
                for kt, (k0, kn) in enumerate(KT):
                    nc.tensor.matmul(out=py[:], lhsT=xrs[kt][:kn, nsl],
                                     rhs=w1_sb[kt][:kn, :],
                                     start=(kt == 0), stop=False)
                    nc.tensor.matmul(out=py[:], lhsT=xrs[kt][:kn, nsl],
                                     rhs=w2_sb[kt][:kn, :],
                                     start=False, stop=False)
                    nc.tensor.matmul(out=py[:], lhsT=xes[kt][:kn, nsl],
                                     rhs=w1_sb[kt][:kn, :],
                                     start=False, stop=(kt == n_kt - 1))
                # yl split to exact (hi, lo) f32r pair -> DRAM
                yl_t = ylpool.tile([128, 2 * NHID], f32r)
                nc.vector.tensor_copy(out=yl_t[:, 0:NHID], in_=py[:, 0:NHID])
                nc.vector.tensor_tensor(out=yl_t[:, NHID:2 * NHID],
                                        in0=py[:, 0:NHID],
                                        in1=yl_t[:, 0:NHID].bitcast(f32),
                                        op=sub_op)
                nc.sync.dma_start(out=ylg[s][nsl, :], in_=yl_t[:])
                yr_t = yrpool.tile([128, NHID], f32)
                nc.vector.tensor_tensor(out=yr_t[:], in0=py[:, NHID:2 * NHID],
                                        in1=bl_sb[:, 0:NHID], op=add_op)
                yr_tiles.append(yr_t)

            # ---- phase 2: exact mean-aggregation, h0 = relu(...) ----
            for ntl in range(TPG):
                d = s * TPG + ntl
                ne = n_et[d]
                c0 = col_start[d]
                ht = hpool.tile([128, NHID], f32)
                if ne > 0:
                    pa = apsum.tile([128, 2 * NHID], f32)
                    for j in range(ne):
                        gt = gpool.tile([128, 2 * NHID], f32r)
                        nc.gpsimd.indirect_dma_start(
                            out=gt[:],
                            out_offset=None,
                            in_=ylg[s][:, :],
                            in_offset=bass.IndirectOffsetOnAxis(
                                ap=src_sb[:, c0 + j:c0 + j + 1], axis=0),
                        )
                        st = spool.tile([128, 128], f32r)
                        nc.vector.tensor_tensor(
                            out=st[:],
                            in0=drel_sb[:, c0 + j:c0 + j + 1].to_broadcast(
                                [128, 128]),
                            in1=iota_sb[:],
                            op=eq_op,
                        )
                        nc.tensor.matmul(out=pa[:], lhsT=st[:], rhs=gt[:],
                                         start=(j == 0), stop=(j == ne - 1))
                    # mean = (aggH + aggL) * recip, one PSUM operand per op
                    nc.vector.tensor_scalar(
                        out=ht[:], in0=pa[:, 0:NHID],
                        scalar1=recip_sb[:, d:d + 1],
                        scalar2=None, op0=mult_op)
                    tmp2 = spool.tile([128, NHID], f32, tag="aggl")
                    nc.vector.tensor_scalar(
                        out=tmp2[:], in0=pa[:, NHID:2 * NHID],
                        scalar1=recip_sb[:, d:d + 1],
                        scalar2=None, op0=mult_op)
                    nc.vector.tensor_tensor(out=ht[:], in0=ht[:],
                                            in1=tmp2[:], op=add_op)
                    nc.vector.tensor_tensor(out=ht[:], in0=ht[:],
                                            in1=yr_tiles[ntl][:], op=add_op)
                else:
                    nc.vector.tensor_copy(out=ht[:], in_=yr_tiles[ntl][:])
                nc.vector.tensor_scalar_max(out=ht[:], in0=ht[:], scalar1=0.0)
                nc.sync.dma_start(out=H0[d * 128:(d + 1) * 128, :], in_=ht[:])

    nc.compile()
    return nc


# --------------------------------------------------------------------------
# host orchestration
# --------------------------------------------------------------------------

def _prep_and_run(x, edge_index, prm):
    """Shard, pack edge streams, run the device program, return h0 full."""
    if _TRN_REPO not in sys.path:
        sys.path.insert(0, _TRN_REPO)
    from concourse.bass_utils import run_bass_kernel_spmd

    global _last_exec_time_ns
    f32 = np.float32
    src = edge_index[0].astype(np.int64)
    dst = edge_index[1].astype(np.int64)

    # fold the first-12-column normalization into the layer-0 weights
    colmax = x[:, :12].max(axis=0)
    wl = prm["c0_wl"].copy()
    wr = prm["c0_wr"].copy()
    wl[:12] = wl[:12] / colmax[:, None]
    wr[:12] = wr[:12] / colmax[:, None]
    Wc = np.ascontiguousarray(np.concatenate([wl, wr], axis=1), f32)

    g_src = src // N0
    g_dst = dst // N0

    # core/slot layout
    slot_of = (np.arange(G) % SLOTS)
    core_of = (np.arange(G) // SLOTS)

    e_core = core_of[g_dst]
    e_slot = slot_of[g_dst]
    dst_in = dst % N0
    src_in = src % N0
    e_tile = e_slot * TPG + dst_in // 128     # dst-tile within core [0, 56)
    e_drel = (dst_in % 128).astype(f32)

    # per-(core, tile) edge counts -> shared edge-tile budgets
    bins = e_core * NTILES + e_tile
    cnt_ct = np.bincount(bins, minlength=NCORES * NTILES).reshape(NCORES, NTILES)
    n_et = ((cnt_ct + 127) // 128).max(axis=0)          # [NTILES]
    col_start = np.zeros(NTILES, np.int64)
    col_start[1:] = np.cumsum(n_et)[:-1]
    et_total = int(n_et.sum())

    # pack per-core SRC/DREL streams: [128, ET] layout
    order = np.lexsort((e_tile, e_core))
    src_srt, drel_srt = src_in[order], e_drel[order]
    bins_srt = bins[order]
    starts = np.zeros(NCORES * NTILES + 1, np.int64)
    starts[1:] = np.cumsum(np.bincount(bins_srt, minlength=NCORES * NTILES))

    SRCs, DRELs, RECIPs, XTs = [], [], [], []
    cnt_node = np.bincount(e_core * M_PAD + (e_slot * NPG + dst_in),
                           minlength=NCORES * M_PAD).reshape(NCORES, M_PAD)
    recip_all = (1.0 / np.maximum(cnt_node, 1)).astype(f32)

    xt = np.ascontiguousarray(x.T)  # [FEAT, G*N0]
    for c in range(NCORES):
        src_flat = np.zeros(et_total * 128, np.int32)
        drel_flat = np.full(et_total * 128, -1.0, f32)
        for d in range(NTILES):
            b = c * NTILES + d
            e0, e1 = starts[b], starts[b + 1]
            if e1 > e0:
                o = col_start[d] * 128
                src_flat[o:o + (e1 - e0)] = src_srt[e0:e1]
                drel_flat[o:o + (e1 - e0)] = drel_srt[e0:e1]
        SRCs.append(np.ascontiguousarray(
            src_flat.reshape(max(et_total, 1), 128).T))
        DRELs.append(np.ascontiguousarray(
            drel_flat.reshape(max(et_total, 1), 128).T))
        RECIPs.append(np.ascontiguousarray(
            recip_all[c].reshape(NTILES, 128).T))
        xtc = np.zeros((FEAT, M_PAD), f32)
        for sl in range(SLOTS):
            g = c * SLOTS + sl
            if g < G:
                xtc[:, sl * NPG:sl * NPG + N0] = xt[:, g * N0:(g + 1) * N0]
        XTs.append(xtc)

    iota = np.ascontiguousarray(
        np.tile(np.arange(128, dtype=f32), (128, 1)))
    bl = np.ascontiguousarray(np.tile(prm["c0_bl"][None, :128], (128, 1)).astype(f32))
    blp = np.zeros((128, 128), f32)
    blp[:, :NHID] = bl[:, :NHID]

    key = (tuple(int(v) for v in n_et),)
    if key not in _nc_cache:
        _nc_cache[key] = _build_nc(
            [int(v) for v in n_et], [int(v) for v in col_start], et_total)
    nc = _nc_cache[key]

    in_maps = []
    for c in range(NCORES):
        in_maps.append({
            "xT": XTs[c], "Wc": Wc, "SRC": SRCs[c], "DREL": DRELs[c],
            "RECIP": RECIPs[c], "IOTA": iota, "BL": blp,
        })

    trace = os.environ.get("KERNEL_TRACE", "0") == "1"
    if trace:
        trace = _install_ntff_hook()
    if trace:
        try:
            res = run_bass_kernel_spmd(nc, in_maps,
                                       core_ids=list(range(NCORES)),
                                       trace=True)
        except Exception:
            import traceback
            traceback.print_exc()
            res = run_bass_kernel_spmd(nc, in_maps,
                                       core_ids=list(range(NCORES)),
                                       trace=False)
    else:
        res = run_bass_kernel_spmd(nc, in_maps, core_ids=list(range(NCORES)),
                                   trace=False)
    _last_exec_time_ns = getattr(res, "exec_time_ns", None)

    h0 = np.empty((G * N0, NHID), f32)
    for c in range(NCORES):
        out = res.results[c]["H0"]
        for sl in range(SLOTS):
            g = c * SLOTS + sl
            if g < G:
                h0[g * N0:(g + 1) * N0] = out[sl * NPG:sl * NPG + N0]
    return h0


def kernel(x, edge_index, graphs_per_pat, **prm):
    x = np.asarray(x, np.float32)
    edge_index = np.asarray(edge_index)
    prm = {k: np.asarray(v, np.float32) for k, v in prm.items()}
    src = edge_index[0].astype(np.int64)
    dst = edge_index[1].astype(np.int64)

    ok = (
        x.shape == (G * N0, FEAT)
        and edge_index.shape[0] == 2
        and src.min() >= 0 and src.max() < G * N0
        and dst.min() >= 0 and dst.max() < G * N0
        and bool(np.all(src // N0 == dst // N0))  # graph-local edges
    )
    if ok:
        try:
            h0 = _prep_and_run(x, edge_index, prm)
        except Exception:
            import traceback
            traceback.print_exc()
            h0 = _layer0_host(x, src, dst, prm)
    else:
        h0 = _layer0_host(x, src, dst, prm)
    return _finish_from_h0(h0, src, dst, graphs_per_pat, prm)


# revision 8
# speedup vs baseline: 1.0108x; 1.0108x over previous
"""Trainium2 Bass kernel for a 3-layer GraphSAGE+SAGPool GNN.

Strategy: the dominant cost (~97% of FLOPs and memory traffic) is layer 0,
which touches x [50000, 1036] (207MB).  By linearity of matmul over the
mean-aggregation, we compute y = x @ [wl|wr] FIRST (one fused 1036->256
matmul), then aggregate in 128-dim space via indirect-DMA gather + one-hot
scatter-matmuls.  The device outputs h0 = relu(mean@wl + bl + x@wr) for all
nodes; the host (numpy) finishes the cheap remainder (SAG pooling, layers
1-2 on <=10k nodes, readouts, patient pooling, dense heads).

Sharding: data-parallel over graphs.  8 cores x 7 graph slots of 1024
padded nodes each.  Each graph's edges are local to its core (checked; a
pure-numpy fallback handles arbitrary inputs).
"""

import os
import sys

import numpy as np

# ---- problem constants (hardcoded per harness contract) ----
G, N0, P = 50, 1000, 10
FEAT, NHID, GDIM, RATIO = 1036, 128, 32, 0.2
NLAYERS = 3
NCORES = 8
SLOTS = 7            # graph slots per core (7*8 = 56 >= 50)
NPG = 1024           # padded nodes per graph slot
TPG = NPG // 128     # node-tiles per graph slot (8)
M_PAD = SLOTS * NPG  # padded nodes per core (7168)
NTILES = SLOTS * TPG # node-tiles per core (56)

_TRN_REPO = "/opt/trn_rl_repo"

_nc_cache = {}
_last_exec_time_ns = None


# --------------------------------------------------------------------------
# host-side numpy ops mirroring the jax reference
# --------------------------------------------------------------------------

def _relu(v):
    return np.maximum(v, np.float32(0.0))


def _log_softmax(v):
    m = v.max(axis=1, keepdims=True)
    s = v - m
    return s - np.log(np.exp(s).sum(axis=1, keepdims=True))


def _sigmoid(v):
    return 1.0 / (1.0 + np.exp(-v))


def _topk_idx(score, k):
    # lax.top_k: descending values, ties -> smaller index first
    return np.argsort(-score, axis=1, kind="stable")[:, :k]


def _finish_from_h0(h0, src, dst, graphs_per_pat, prm):
    """Everything after the device-computed layer-0 SAGE output h0."""
    f32 = np.float32
    x = h0.astype(f32, copy=False)
    n = N0
    src_c = src.astype(np.int64, copy=False)
    dst_c = dst.astype(np.int64, copy=False)
    xs = []
    for l in range(NLAYERS):
        k = int(np.ceil(RATIO * n))
        M = G * n
        if l > 0:
            # SAGE (mean aggr) on compacted surviving edges
            yl = x @ prm[f"c{l}_wl"]
            yr = x @ prm[f"c{l}_wr"]
            agg = np.zeros((M, NHID), f32)
            np.add.at(agg, dst_c, yl[src_c])
            cnt = np.bincount(dst_c, minlength=M).astype(f32)
            x = _relu(agg / np.maximum(cnt, 1.0)[:, None] + prm[f"c{l}_bl"] + yr)
        # SAGPool: GraphConv (add aggr) score -> per-graph top-k
        s = x @ prm[f"p{l}_wrel"]  # [M, 1]
        aggs = np.zeros((M, 1), f32)
        np.add.at(aggs, dst_c, s[src_c])
        score = (aggs + prm[f"p{l}_brel"] + x @ prm[f"p{l}_wroot"]).reshape(G, n)
        idx = _topk_idx(score, k)
        sel = np.take_along_axis(score, idx, axis=1)
        perm = (idx + (np.arange(G) * n)[:, None]).reshape(-1)
        x = x[perm] * np.tanh(sel).reshape(-1, 1).astype(f32)
        mapping = np.full(M, -1, np.int64)
        mapping[perm] = np.arange(G * k)
        ns, nd = mapping[src_c], mapping[dst_c]
        keep = (ns >= 0) & (nd >= 0)
        src_c, dst_c = ns[keep], nd[keep]
        n = k
        xr = x.reshape(G, n, NHID)
        xs.append(np.concatenate([xr.max(axis=1), xr.mean(axis=1)], axis=1))
    h = xs[0] + xs[1] + xs[2]  # [G, 2*NHID]

    # patient-level mean pool (jnp.repeat with total_repeat_length=G)
    gpp = np.asarray(graphs_per_pat).astype(np.int64)
    bv = np.repeat(np.arange(P), np.maximum(gpp, 0))
    if len(bv) >= G:
        bv = bv[:G]
    else:
        pad_val = bv[-1] if len(bv) else 0
        bv = np.concatenate([bv, np.full(G - len(bv), pad_val, np.int64)])
    pooled = np.zeros((P, 2 * NHID), f32)
    np.add.at(pooled, bv, h)
    with np.errstate(divide="ignore", invalid="ignore"):
        pooled = pooled / gpp[:, None].astype(f32)

    h1 = _relu(pooled @ prm["lin1_w"] + prm["lin1_b"])
    feats = _relu(h1 @ prm["lin2_w"] + prm["lin2_b"])
    grade = _log_softmax(feats @ prm["grade_w"] + prm["grade_b"])
    hazard = _sigmoid(feats @ prm["haz_w"] + prm["haz_b"]) * 6.0 - 3.0
    return (
        feats.astype(f32),
        grade.astype(f32),
        hazard.astype(f32),
    )


def _layer0_host(x, src, dst, prm):
    """Pure-numpy layer-0 fallback (arbitrary edge structure)."""
    f32 = np.float32
    xn = x.copy()
    xn[:, :12] = xn[:, :12] / xn[:, :12].max(axis=0, keepdims=True)
    yl = xn @ prm["c0_wl"]
    yr = xn @ prm["c0_wr"]
    M = xn.shape[0]
    agg = np.zeros((M, NHID), f32)
    np.add.at(agg, dst.astype(np.int64), yl[src.astype(np.int64)])
    cnt = np.bincount(dst.astype(np.int64), minlength=M).astype(f32)
    return _relu(agg / np.maximum(cnt, 1.0)[:, None] + prm["c0_bl"] + yr)


def _install_ntff_hook():
    """Register the axon NTFF profiling hook (missing antenv.axon_hooks)."""
    import contextlib
    import ctypes
    import types

    if "antenv.axon_hooks" in sys.modules:
        return True
    try:
        import antenv
    except ImportError:
        return False
    so_path = "/opt/axon/libaxon_pjrt.so"
    if not os.path.exists(so_path):
        return False
    lib = ctypes.CDLL(so_path)
    if not hasattr(lib, "axon_start_nrt_profile"):
        return False
    lib.axon_start_nrt_profile.argtypes = [
        ctypes.POINTER(ctypes.c_int64), ctypes.c_size_t]
    lib.axon_start_nrt_profile.restype = ctypes.c_int64
    lib.axon_stop_nrt_profile.argtypes = [ctypes.c_char_p]
    lib.axon_stop_nrt_profile.restype = ctypes.c_int64

    @contextlib.contextmanager
    def _hook(output_dir, device_ids):
        import jax

        jax.devices()
        if device_ids:
            ids = (ctypes.c_int64 * len(device_ids))(*device_ids)
            rc = lib.axon_start_nrt_profile(ids, len(device_ids))
        else:
            rc = lib.axon_start_nrt_profile(None, 0)
        if rc != 0:
            raise RuntimeError(f"axon_start_nrt_profile rc={rc}")
        try:
            yield
        finally:
            n = lib.axon_stop_nrt_profile(str(output_dir).encode())
            print(f"ntff profile: {n} file(s) -> {output_dir}", file=sys.stderr)

    mod = types.ModuleType("antenv.axon_hooks")
    mod.get_axon_ntff_profile_hook = lambda: _hook
    mod.set_axon_ntff_profile_hook = lambda h: None
    sys.modules["antenv.axon_hooks"] = mod
    antenv.axon_hooks = mod
    return True


# --------------------------------------------------------------------------
# device program
# --------------------------------------------------------------------------

def _build_nc(n_et, col_start, et_total):
    if _TRN_REPO not in sys.path:
        sys.path.insert(0, _TRN_REPO)
    from contextlib import ExitStack

    from concourse import bacc, bass, mybir, tile

    f32 = mybir.dt.float32
    f16 = mybir.dt.float16
    f32r = mybir.dt.float32r
    i32 = mybir.dt.int32
    add_op = mybir.AluOpType.add
    sub_op = mybir.AluOpType.subtract
    eq_op = mybir.AluOpType.is_equal
    mult_op = mybir.AluOpType.mult

    ET = max(et_total, 1)
    nc = bacc.Bacc("TRN2", target_bir_lowering=False, debug=False,
                   num_devices=NCORES)
    xT = nc.dram_tensor("xT", [FEAT, M_PAD], f32, kind="ExternalInput").ap()
    Wc = nc.dram_tensor("Wc", [FEAT, 2 * NHID], f32, kind="ExternalInput").ap()
    SRC = nc.dram_tensor("SRC", [128, ET], i32, kind="ExternalInput").ap()
    DREL = nc.dram_tensor("DREL", [128, ET], f32, kind="ExternalInput").ap()
    RECIP = nc.dram_tensor("RECIP", [128, NTILES], f32, kind="ExternalInput").ap()
    IOTA = nc.dram_tensor("IOTA", [128, 128], f32, kind="ExternalInput").ap()
    BL = nc.dram_tensor("BL", [128, 128], f32, kind="ExternalInput").ap()
    H0 = nc.dram_tensor("H0", [M_PAD, NHID], f32, kind="ExternalOutput").ap()
    # per-graph-slot yl table: [node, 0:128]=hi (f32r), [node, 128:256]=lo
    ylg = [nc.dram_tensor(f"yl{s}", [NPG, 2 * NHID], f32r) for s in range(SLOTS)]

    n_kt = (FEAT + 127) // 128
    KT = [(kt * 128, min(128, FEAT - kt * 128)) for kt in range(n_kt)]

    with tile.TileContext(nc) as tc, ExitStack() as ctx:
        cpool = ctx.enter_context(tc.tile_pool(name="consts", bufs=1))
        wfpool = ctx.enter_context(tc.tile_pool(name="wf", bufs=2))
        wpool = ctx.enter_context(tc.tile_pool(name="w", bufs=2 * n_kt))
        xfpool = ctx.enter_context(tc.tile_pool(name="xf", bufs=5))
        xrpool = ctx.enter_context(tc.tile_pool(name="xr", bufs=2 * n_kt))
        xepool = ctx.enter_context(tc.tile_pool(name="xe", bufs=2 * n_kt))
        ypsum = ctx.enter_context(tc.tile_pool(name="ypsum", bufs=4, space="PSUM"))
        apsum = ctx.enter_context(tc.tile_pool(name="apsum", bufs=4, space="PSUM"))
        ysb = ctx.enter_context(tc.tile_pool(name="ysb", bufs=4))
        ylpool = ctx.enter_context(tc.tile_pool(name="ylt", bufs=4))
        yrpool = ctx.enter_context(tc.tile_pool(name="yrb", bufs=2 * TPG))
        gpool = ctx.enter_context(tc.tile_pool(name="gath", bufs=16))
        spool = ctx.enter_context(tc.tile_pool(name="sel", bufs=16))
        hpool = ctx.enter_context(tc.tile_pool(name="h0t", bufs=8))

        iota_sb = cpool.tile([128, 128], f32)
        nc.sync.dma_start(out=iota_sb[:], in_=IOTA[:])
        bl_sb = cpool.tile([128, 128], f32)
        nc.sync.dma_start(out=bl_sb[:], in_=BL[:])
        recip_sb = cpool.tile([128, NTILES], f32)
        nc.sync.dma_start(out=recip_sb[:], in_=RECIP[:])
        src_sb = cpool.tile([128, ET], i32)
        nc.sync.dma_start(out=src_sb[:], in_=SRC[:])
        drel_sb = cpool.tile([128, ET], f32)
        nc.sync.dma_start(out=drel_sb[:], in_=DREL[:])

        # W split: w = w1 + w2 (fp16 hi + fp16 residual), exact to ~2^-22
        w1_sb = []
        w2_sb = []
        for k0, kn in KT:
            wf = wfpool.tile([128, 2 * NHID], f32)
            nc.sync.dma_start(out=wf[:kn, :], in_=Wc[k0:k0 + kn, :])
            t1 = wpool.tile([128, 2 * NHID], f16, tag="w1")
            nc.vector.tensor_copy(out=t1[:kn, :], in_=wf[:kn, :])
            t2 = wpool.tile([128, 2 * NHID], f16, tag="w2")
            nc.vector.tensor_tensor(out=t2[:kn, :], in0=wf[:kn, :],
                                    in1=t1[:kn, :], op=sub_op)
            w1_sb.append(t1)
            w2_sb.append(t2)

        for s in range(SLOTS):
            # ---- phase 1: y = x @ [wl|wr] exactly via f32r split ----
            xrs, xes = [], []
            for kt, (k0, kn) in enumerate(KT):
                xf = xfpool.tile([128, NPG], f32)
                nc.sync.dma_start(out=xf[:kn, :],
                                  in_=xT[k0:k0 + kn, s * NPG:(s + 1) * NPG])
                xr = xrpool.tile([128, NPG], f16)
                nc.vector.tensor_copy(out=xr[:kn, :], in_=xf[:kn, :])
                xe = xepool.tile([128, NPG], f16)
                nc.vector.tensor_tensor(out=xe[:kn, :], in0=xf[:kn, :],
                                        in1=xr[:kn, :], op=sub_op)
                xrs.append(xr)
                xes.append(xe)
            yr_tiles = []
            for ntl in range(TPG):
                nsl = slice(ntl * 128, (ntl + 1) * 128)
                py = ypsum.tile([128, 2 * NHID], f32)
                for kt, (k0, kn) in enumerate(KT):
                    nc.tensor.matmul(out=py[:], lhsT=xrs[kt][:kn, nsl],
                                     rhs=w1_sb[kt][:kn, :],
                                     start=(kt == 0), stop=False)
                    nc.tensor.matmul(out=py[:], lhsT=xrs[kt][:kn, nsl],
                                     rhs=w2_sb[kt][:kn, :],
                                     start=False, stop=False)
                    nc.tensor.matmul(out=py[:], lhsT=xes[kt][:kn, nsl],
                                     rhs=w1_sb[kt][:kn, :],
                                     start=False, stop=(kt == n_kt - 1))
                # yl split to exact (hi, lo) f32r pair -> DRAM
                yl_t = ylpool.tile([128, 2 * NHID], f32r)
                nc.vector.tensor_copy(out=yl_t[:, 0:NHID], in_=py[:, 0:NHID])
                nc.vector.tensor_tensor(out=yl_t[:, NHID:2 * NHID],
                                        in0=py[:, 0:NHID],
                                        in1=yl_t[:, 0:NHID].bitcast(f32),
                                        op=sub_op)
                nc.sync.dma_start(out=ylg[s][nsl, :], in_=yl_t[:])
                yr_t = yrpool.tile([128, NHID], f32)
                nc.vector.tensor_tensor(out=yr_t[:], in0=py[:, NHID:2 * NHID],
                                        in1=bl_sb[:, 0:NHID], op=add_op)
                yr_tiles.append(yr_t)

            # ---- phase 2: exact mean-aggregation, h0 = relu(...) ----
            for ntl in range(TPG):
                d = s * TPG + ntl
                ne = n_et[d]
                c0 = col_start[d]
                ht = hpool.tile([128, NHID], f32)
                if ne > 0:
                    pa = apsum.tile([128, 2 * NHID], f32)
                    for j in range(ne):
                        gt = gpool.tile([128, 2 * NHID], f32r)
                        nc.gpsimd.indirect_dma_start(
                            out=gt[:],
                            out_offset=None,
                            in_=ylg[s][:, :],
                            in_offset=bass.IndirectOffsetOnAxis(
                                ap=src_sb[:, c0 + j:c0 + j + 1], axis=0),
                        )
                        st = spool.tile([128, 128], f32r)
                        nc.vector.tensor_tensor(
                            out=st[:],
                            in0=drel_sb[:, c0 + j:c0 + j + 1].to_broadcast(
                                [128, 128]),
                            in1=iota_sb[:],
                            op=eq_op,
                        )
                        nc.tensor.matmul(out=pa[:], lhsT=st[:], rhs=gt[:],
                                         start=(j == 0), stop=(j == ne - 1))
                    # mean = (aggH + aggL) * recip, one PSUM operand per op
                    nc.vector.tensor_scalar(
                        out=ht[:], in0=pa[:, 0:NHID],
                        scalar1=recip_sb[:, d:d + 1],
                        scalar2=None, op0=mult_op)
                    tmp2 = spool.tile([128, NHID], f32, tag="aggl")
                    nc.vector.tensor_scalar(
                        out=tmp2[:], in0=pa[:, NHID:2 * NHID],
                        scalar1=recip_sb[:, d:d + 1],
                        scalar2=None, op0=mult_op)
                    nc.vector.tensor_tensor(out=ht[:], in0=ht[:],
                                            in1=tmp2[:], op=add_op)
                    nc.vector.tensor_tensor(out=ht[:], in0=ht[:],
                                            in1=yr_tiles[ntl][:], op=add_op)
                else:
                    nc.vector.tensor_copy(out=ht[:], in_=yr_tiles[ntl][:])
                nc.vector.tensor_scalar_max(out=ht[:], in0=ht[:], scalar1=0.0)
                nc.sync.dma_start(out=H0[d * 128:(d + 1) * 128, :], in_=ht[:])

    nc.compile()
    return nc


# --------------------------------------------------------------------------
# host orchestration
# --------------------------------------------------------------------------

def _prep_and_run(x, edge_index, prm):
    """Shard, pack edge streams, run the device program, return h0 full."""
    if _TRN_REPO not in sys.path:
        sys.path.insert(0, _TRN_REPO)
    from concourse.bass_utils import run_bass_kernel_spmd

    global _last_exec_time_ns
    f32 = np.float32
    src = edge_index[0].astype(np.int64)
    dst = edge_index[1].astype(np.int64)

    # fold the first-12-column normalization into the layer-0 weights
    colmax = x[:, :12].max(axis=0)
    wl = prm["c0_wl"].copy()
    wr = prm["c0_wr"].copy()
    wl[:12] = wl[:12] / colmax[:, None]
    wr[:12] = wr[:12] / colmax[:, None]
    Wc = np.ascontiguousarray(np.concatenate([wl, wr], axis=1), f32)

    g_src = src // N0
    g_dst = dst // N0

    # core/slot layout
    slot_of = (np.arange(G) % SLOTS)
    core_of = (np.arange(G) // SLOTS)

    e_core = core_of[g_dst]
    e_slot = slot_of[g_dst]
    dst_in = dst % N0
    src_in = src % N0
    e_tile = e_slot * TPG + dst_in // 128     # dst-tile within core [0, 56)
    e_drel = (dst_in % 128).astype(f32)

    # per-(core, tile) edge counts -> shared edge-tile budgets
    bins = e_core * NTILES + e_tile
    cnt_ct = np.bincount(bins, minlength=NCORES * NTILES).reshape(NCORES, NTILES)
    n_et = ((cnt_ct + 127) // 128).max(axis=0)          # [NTILES]
    col_start = np.zeros(NTILES, np.int64)
    col_start[1:] = np.cumsum(n_et)[:-1]
    et_total = int(n_et.sum())

    # pack per-core SRC/DREL streams: [128, ET] layout
    order = np.lexsort((e_tile, e_core))
    src_srt, drel_srt = src_in[order], e_drel[order]
    bins_srt = bins[order]
    starts = np.zeros(NCORES * NTILES + 1, np.int64)
    starts[1:] = np.cumsum(np.bincount(bins_srt, minlength=NCORES * NTILES))

    SRCs, DRELs, RECIPs, XTs = [], [], [], []
    cnt_node = np.bincount(e_core * M_PAD + (e_slot * NPG + dst_in),
                           minlength=NCORES * M_PAD).reshape(NCORES, M_PAD)
    recip_all = (1.0 / np.maximum(cnt_node, 1)).astype(f32)

    xt = np.ascontiguousarray(x.T)  # [FEAT, G*N0]
    for c in range(NCORES):
        src_flat = np.zeros(et_total * 128, np.int32)
        drel_flat = np.full(et_total * 128, -1.0, f32)
        for d in range(NTILES):
            b = c * NTILES + d
            e0, e1 = starts[b], starts[b + 1]
            if e1 > e0:
                o = col_start[d] * 128
                src_flat[o:o + (e1 - e0)] = src_srt[e0:e1]
                drel_flat[o:o + (e1 - e0)] = drel_srt[e0:e1]
        SRCs.append(np.ascontiguousarray(
            src_flat.reshape(max(et_total, 1), 128).T))
        DRELs.append(np.ascontiguousarray(
            drel_flat.reshape(max(et_total, 1), 128).T))
        RECIPs.append(np.ascontiguousarray(
            recip_all[c].reshape(NTILES, 128).T))
        xtc = np.zeros((FEAT, M_PAD), f32)
        for sl in range(SLOTS):
            g = c * SLOTS + sl
            if g < G:
                xtc[:, sl * NPG:sl * NPG + N0] = xt[:, g * N0:(g + 1) * N0]
        XTs.append(xtc)

    iota = np.ascontiguousarray(
        np.tile(np.arange(128, dtype=f32), (128, 1)))
    bl = np.ascontiguousarray(np.tile(prm["c0_bl"][None, :128], (128, 1)).astype(f32))
    blp = np.zeros((128, 128), f32)
    blp[:, :NHID] = bl[:, :NHID]

    key = (tuple(int(v) for v in n_et),)
    if key not in _nc_cache:
        _nc_cache[key] = _build_nc(
            [int(v) for v in n_et], [int(v) for v in col_start], et_total)
    nc = _nc_cache[key]

    in_maps = []
    for c in range(NCORES):
        in_maps.append({
            "xT": XTs[c], "Wc": Wc, "SRC": SRCs[c], "DREL": DRELs[c],
            "RECIP": RECIPs[c], "IOTA": iota, "BL": blp,
        })

    trace = os.environ.get("KERNEL_TRACE", "0") == "1"
    if trace:
        trace = _install_ntff_hook()
    if trace:
        try:
            res = run_bass_kernel_spmd(nc, in_maps,
                                       core_ids=list(range(NCORES)),
                                       trace=True)
        except Exception:
            import traceback
            traceback.print_exc()
            res = run_bass_kernel_spmd(nc, in_maps,
                                       core_ids=list(range(NCORES)),
                                       trace=False)
    else:
        res = run_bass_kernel_spmd(nc, in_maps, core_ids=list(range(NCORES)),
                                   trace=False)
    _last_exec_time_ns = getattr(res, "exec_time_ns", None)

    h0 = np.empty((G * N0, NHID), f32)
    for c in range(NCORES):
        out = res.results[c]["H0"]
        for sl in range(SLOTS):
            g = c * SLOTS + sl
            if g < G:
                h0[g * N0:(g + 1) * N0] = out[sl * NPG:sl * NPG + N0]
    return h0


def kernel(x, edge_index, graphs_per_pat, **prm):
    x = np.asarray(x, np.float32)
    edge_index = np.asarray(edge_index)
    prm = {k: np.asarray(v, np.float32) for k, v in prm.items()}
    src = edge_index[0].astype(np.int64)
    dst = edge_index[1].astype(np.int64)

    ok = (
        x.shape == (G * N0, FEAT)
        and edge_index.shape[0] == 2
        and src.min() >= 0 and src.max() < G * N0
        and dst.min() >= 0 and dst.max() < G * N0
        and bool(np.all(src // N0 == dst // N0))  # graph-local edges
    )
    if ok:
        try:
            h0 = _prep_and_run(x, edge_index, prm)
        except Exception:
            import traceback
            traceback.print_exc()
            h0 = _layer0_host(x, src, dst, prm)
    else:
        h0 = _layer0_host(x, src, dst, prm)
    return _finish_from_h0(h0, src, dst, graphs_per_pat, prm)
